# revision 14
# baseline (speedup 1.0000x reference)
"""CapsuleLayer (B=32, J=32, I=2048, T=16, D=16, 3 routing iters) on 8 TRN2 cores.

Strategy: shard input-capsule axis I across the 8 cores (I_loc = 256).
W reads stay at the 8.4 MB/core roofline; all routing state except the
tiny s[b,j,t] (64 KB, AllReduce x3) is core-local.

Per core:
  - u_hat computed on TensorE: K=(i_sub 4, d 16)=64, M=(i_sub 4, b 32)=128,
    N=(j,t)=512, with host-prepacked block-diagonal x weights (float32r) and
    W streamed as float32r rhs at 1 cyc/row.
  - s0 partial (uniform-c weighted sum) fused into the same W pass as one
    extra K=128 matmul per W tile accumulating into a single PSUM bank.
  - u_hat stored once in SBUF as bf16, layout [part=(i_sub,b), free=(g,t,j)].
  - routing: DVE bf16 2x-mode multiplies; first tree level on GpSimd;
    ACT exp; TensorE delta_b-ones restream (PSUM accumulation) for the
    i-sums; AllReduce x3 for the cross-core sum_i.
  - collective warm-up + ACT table preloads issued at t=0 so the first
    real AllReduce only pays steady-state latency.
"""

import functools
import os
import sys

import numpy as np

sys.path.insert(0, "/opt/trn_rl_repo")

import ml_dtypes  # noqa: E402

import concourse.bass as bass  # noqa: E402
import concourse.bacc as bacc  # noqa: E402
import concourse.mybir as mybir  # noqa: E402
import concourse.tile as tile  # noqa: E402

F32 = mybir.dt.float32
F32R = mybir.dt.float32r
BF16 = mybir.dt.bfloat16

NCORES = 8
B, J, I, T, D = 32, 32, 2048, 16, 16
ILOC = I // NCORES          # 256
G = ILOC // 4               # 64 i-groups of 4
EPS = 1e-9

NWT = G // 2                # 32 w dram tiles, each [128, 512] covers 2 g
WCH = 4                     # w tiles per DMA chunk
NCHD = NWT // WCH           # 8 dma chunks


def _build_program(single=False):
    nc = bacc.Bacc(
        "TRN2",
        target_bir_lowering=False,
        debug=False,
        enable_asserts=False,
        num_devices=1 if single else NCORES,
    )

    wt_d = nc.dram_tensor("wt", [NWT, 128, 512], F32R, kind="ExternalInput")
    xd_d = nc.dram_tensor("xd", [NWT, 128, 128], F32R, kind="ExternalInput")
    xp_d = nc.dram_tensor("xplain", [NWT, 128, 32], F32R, kind="ExternalInput")
    warm_d = nc.dram_tensor("warm", [1, 8], F32, kind="ExternalInput")
    ones_d = nc.dram_tensor("onesdb", [128, 32], BF16, kind="ExternalInput")
    repl_d = nc.dram_tensor("repl", [32, 128], BF16, kind="ExternalInput")
    out_d = nc.dram_tensor("outv", [32, 512], F32, kind="ExternalOutput")

    with tile.TileContext(nc) as tc:
        _capsule(
            tc, wt_d.ap(), xd_d.ap(), xp_d.ap(), ones_d.ap(), repl_d.ap(),
            out_d.ap(), warm_d.ap(), single=single,
        )
    nc.compile()
    return nc


def _capsule(tc, wt, xd, xpd, ones_dram, repl_dram, outv, warm_dram, single=False):
    nc = tc.nc
    from contextlib import ExitStack

    ctx = ExitStack()
    with ctx:
        up = ctx.enter_context(tc.tile_pool(name="u", bufs=1))
        wp = ctx.enter_context(tc.tile_pool(name="w", bufs=3))
        xp = ctx.enter_context(tc.tile_pool(name="x", bufs=1))
        cp = ctx.enter_context(tc.tile_pool(name="consts", bufs=1))
        qp = ctx.enter_context(tc.tile_pool(name="q", bufs=4))
        tp = ctx.enter_context(tc.tile_pool(name="tree", bufs=3))
        bp = ctx.enter_context(tc.tile_pool(name="bij", bufs=1))
        sp = ctx.enter_context(tc.tile_pool(name="small", bufs=2))
        vp = ctx.enter_context(tc.tile_pool(name="vexp", bufs=2))
        pup = ctx.enter_context(tc.tile_pool(name="upsum", bufs=4, space="PSUM"))
        psp = ctx.enter_context(tc.tile_pool(name="spsum", bufs=2, space="PSUM"))
        prp = ctx.enter_context(tc.tile_pool(name="rpsum", bufs=2, space="PSUM"))
        dp = ctx.enter_context(tc.tile_pool(name="dram", bufs=6, space="DRAM"))

        # ---- persistent tiles
        u = up.tile([128, G * 512], BF16)          # [(i_sub,b), (g,t,j)]
        xall = xp.tile([128, NWT * 128], F32R)     # all block-diag x weights
        xpl = xp.tile([128, NWT * 32], F32R)       # plain x lhsT per tile
        ones = cp.tile([128, 32], BF16)            # delta_b
        repl = cp.tile([32, 128], BF16)            # v replication matrix
        bijs = [bp.tile([128, 256], F32, tag=f"bij{c}", name=f"bij{c}") for c in range(8)]

        # ---- warm-ups: ACT tables only (first collective cost is intrinsic)
        wact = cp.tile([1, 8], F32, name="warm_act")
        wsb = cp.tile([1, 8], F32, name="warm_sb")
        nc.vector.memset(wsb[:, :], 0.0)
        nc.scalar.activation(wact[:, :], wsb[:, :], mybir.ActivationFunctionType.Exp)
        nc.scalar.activation(
            wact[:, :], wsb[:, :], mybir.ActivationFunctionType.Sqrt, bias=0.0
        )
        nc.scalar.copy(wact[:, :], wsb[:, :])

        nc.sync.dma_start(ones[:, :], ones_dram)
        nc.sync.dma_start(repl[:, :], repl_dram)
        # xd dram [NWT, 128, 128] -> sbuf [128, (p,m)]; split for subtile deps
        for h in range(4):
            pl, ph = h * 8, (h + 1) * 8
            nc.sync.dma_start(
                xall[:, pl * 128 : ph * 128].rearrange("k (p m) -> k p m", p=8),
                xd[pl:ph].transpose([1, 0, 2]),
            )
            nc.sync.dma_start(
                xpl[:, pl * 32 : ph * 32].rearrange("k (p m) -> k p m", p=8),
                xpd[pl:ph].transpose([1, 0, 2]),
            )

        # ---- phase A: u_hat + fused s0 partial
        cp_engines = [nc.scalar, nc.vector]  # gpsimd cannot access PSUM
        s0acc = sp.tile([32, 512], F32, tag="s0acc", bufs=1)
        ncopy = 0
        for c in range(NCHD):
            wch = wp.tile([128, WCH * 512], F32R, tag="w")
            nc.sync.dma_start(
                wch[:, :].rearrange("k (q m) -> k q m", q=WCH),
                wt[c * WCH : (c + 1) * WCH].transpose([1, 0, 2]),
            )
            for qi in range(WCH):
                p = c * WCH + qi
                w_p = wch[:, qi * 512 : (qi + 1) * 512]
                for gl in range(2):
                    g = 2 * p + gl
                    ups = pup.tile([128, 512], F32)
                    nc.tensor.matmul(
                        ups[:, :],
                        lhsT=xall[gl * 64 : (gl + 1) * 64, p * 128 : (p + 1) * 128],
                        rhs=w_p[gl * 64 : (gl + 1) * 64, :],
                        start=True,
                        stop=True,
                    )
                    # psum free = (j,t) ; u free = (t,j)
                    src = ups[:, :].rearrange("p (j t) -> p t j", j=32)
                    dst = u[:, g * 512 : (g + 1) * 512].rearrange(
                        "p (t j) -> p t j", t=16
                    )
                    eng = cp_engines[ncopy % 2]
                    ncopy += 1
                    if eng is nc.scalar:
                        eng.copy(dst, src)
                    else:
                        eng.tensor_copy(dst, src)
            # s0 partial for this chunk: contiguous PSUM accumulation group
            # (interleaving with start=True u_hat matmuls breaks the group)
            sps0 = psp.tile([32, 512], F32, tag="s")
            for qi in range(WCH):
                p = c * WCH + qi
                w_p = wch[:, qi * 512 : (qi + 1) * 512]
                nc.tensor.matmul(
                    sps0[:, :],
                    lhsT=xpl[:, p * 32 : (p + 1) * 32],
                    rhs=w_p[:, :],
                    start=(qi == 0),
                    stop=(qi == WCH - 1),
                )
            if c == 0:
                nc.scalar.copy(s0acc[:, :], sps0[:, :])
            else:
                nc.vector.tensor_add(s0acc[:, :], s0acc[:, :], sps0[:, :])
        # s0acc free layout is (j,t) [from W]; transpose to (t,j) while scaling
        s0 = sp.tile([32, 512], F32, tag="s_sb")
        nc.scalar.mul(
            s0[:, :].rearrange("p (t j) -> p j t", t=16),
            s0acc[:, :].rearrange("p (j t) -> p j t", j=32),
            1.0 / J,
        )

        vexp = _allreduce_squash(
            tc, dp, sp, prp, vp, repl, s0, r=0, single=single, final=False,
            dbg_out=outv,
        )

        # ---- routing iterations (software-pipelined; chunks 6-7 on GpSimd)
        for r in (1, 2):
            sps = psp.tile([32, 512], F32, tag="s")
            nmm = [0]

            def stage1(ch, eng):
                bij = bijs[ch]
                usl = u[:, ch * 4096 : (ch + 1) * 4096].rearrange(
                    "p (g t j) -> p g t j", g=8, t=16
                )
                q = qp.tile([128, 4096], BF16, tag="q", name=f"q{r}{ch}")
                vb = (
                    vexp[:, :]
                    .rearrange("p (t j) -> p t j", t=16)
                    .unsqueeze(1)
                    .to_broadcast([128, 8, 16, 32])
                )
                eng.tensor_mul(
                    q[:, :].rearrange("p (g t j) -> p g t j", g=8, t=16), usl, vb
                )
                l1 = tp.tile([128, 2048], BF16, tag="l1", name=f"l1{r}{ch}")
                q4 = q[:, :].rearrange("p (g t j) -> p g t j", g=8, t=16)
                eng.tensor_add(
                    l1[:, :].rearrange("p (g t j) -> p g t j", g=8, t=8),
                    q4[:, :, 0:8, :],
                    q4[:, :, 8:16, :],
                )
                l2 = tp.tile([128, 1024], BF16, tag="l2", name=f"l2{r}{ch}")
                l14 = l1[:, :].rearrange("p (g t j) -> p g t j", g=8, t=8)
                eng.tensor_add(
                    l2[:, :].rearrange("p (g t j) -> p g t j", g=8, t=4),
                    l14[:, :, 0:4, :],
                    l14[:, :, 4:8, :],
                )
                l3 = tp.tile([128, 512], BF16, tag="l3", name=f"l3{r}{ch}")
                l24 = l2[:, :].rearrange("p (g t j) -> p g t j", g=8, t=4)
                eng.tensor_add(
                    l3[:, :].rearrange("p (g t j) -> p g t j", g=8, t=2),
                    l24[:, :, 0:2, :],
                    l24[:, :, 2:4, :],
                )
                bsl = bij[:, :].rearrange("p (g j) -> p g j", g=8)
                l3a = l3[:, :].rearrange("p (g t j) -> p g t j", g=8, t=2)
                if r == 1:
                    eng.tensor_add(bsl, l3a[:, :, 0, :], l3a[:, :, 1, :])
                else:
                    dd = tp.tile([128, 256], F32, tag="dd", name=f"dd{r}{ch}")
                    eng.tensor_add(
                        dd[:, :].rearrange("p (g j) -> p g j", g=8),
                        l3a[:, :, 0, :],
                        l3a[:, :, 1, :],
                    )
                    eng.tensor_add(
                        bsl, bsl, dd[:, :].rearrange("p (g j) -> p g j", g=8)
                    )
                cte = tp.tile([128, 256], BF16, tag="cte", name=f"cte{r}{ch}")
                if eng is nc.gpsimd:
                    # per-g exp with ACT accumulate -> z for free (no reduce)
                    zt = tp.tile([128, 8], F32, tag="z", name=f"z{r}{ch}")
                    for gg in range(8):
                        nc.scalar.activation(
                            cte[:, gg * 32 : (gg + 1) * 32],
                            bij[:, gg * 32 : (gg + 1) * 32],
                            mybir.ActivationFunctionType.Exp,
                            accum_out=zt[:, gg : gg + 1],
                        )
                    return cte, zt
                nc.scalar.activation(
                    cte[:, :], bij[:, :], mybir.ActivationFunctionType.Exp
                )
                return cte, None

            def stage2(ch, eng, cte, z, invz):
                usl = u[:, ch * 4096 : (ch + 1) * 4096].rearrange(
                    "p (g t j) -> p g t j", g=8, t=16
                )
                if z is None:
                    z = tp.tile([128, 8], F32, tag="z", name=f"z{r}{ch}")
                    eng.tensor_reduce(
                        z[:, :],
                        cte[:, :].rearrange("p (g j) -> p g j", g=8),
                        mybir.AxisListType.X,
                        mybir.AluOpType.add,
                    )
                cc = tp.tile([128, 256], BF16, tag="cc", name=f"cc{r}{ch}")
                if invz is None:
                    invz = tp.tile([128, 8], F32, tag="invz", name=f"iz{r}{ch}")
                    nc.vector.reciprocal(invz[:, :], z[:, :])
                eng.tensor_mul(
                    cc[:, :].rearrange("p (g j) -> p g j", g=8),
                    cte[:, :].rearrange("p (g j) -> p g j", g=8),
                    invz[:, :].unsqueeze(2).to_broadcast([128, 8, 32]),
                )
                p2 = qp.tile([128, 4096], BF16, tag="q", name=f"p2{r}{ch}")
                ccb = (
                    cc[:, :]
                    .rearrange("p (g j) -> p g j", g=8)
                    .unsqueeze(2)
                    .to_broadcast([128, 8, 16, 32])
                )
                eng.tensor_mul(
                    p2[:, :].rearrange("p (g t j) -> p g t j", g=8, t=16), usl, ccb
                )
                for gl in range(8):
                    nc.tensor.matmul(
                        sps[:, :],
                        lhsT=ones[:, :],
                        rhs=p2[:, gl * 512 : (gl + 1) * 512],
                        start=(nmm[0] == 0),
                        stop=(nmm[0] == G - 1),
                    )
                    nmm[0] += 1

            NV = 6  # chunks on DVE; 6..7 on GpSimd
            ctes = {}
            ctes[0] = stage1(0, nc.vector)
            # Pool chunks start their stage1 immediately on their own engine
            for ch in range(NV, 8):
                ctes[ch] = stage1(ch, nc.gpsimd)
            pool_invz = {}
            for ch in range(NV):
                if ch + 1 < NV:
                    ctes[ch + 1] = stage1(ch + 1, nc.vector)
                stage2(ch, nc.vector, *ctes[ch], None)
                # interleave Pool-chunk reciprocals into the DVE stream late
                # enough that Pool's z is ready (no head-of-line stall)
                if ch == 2 or ch == 4:
                    pc = NV + (0 if ch == 2 else 1)
                    zt = ctes[pc][1]
                    iv = tp.tile([128, 8], F32, tag="invz", name=f"iz{r}{pc}")
                    nc.vector.reciprocal(iv[:, :], zt[:, :])
                    pool_invz[pc] = iv
            for ch in range(NV, 8):
                stage2(ch, nc.gpsimd, ctes[ch][0], ctes[ch][1], pool_invz[ch])
            ssb = sp.tile([32, 512], F32, tag="s_sb")
            nc.scalar.copy(ssb[:, :], sps[:, :])

            vexp = _allreduce_squash(
                tc, dp, sp, prp, vp, repl, ssb, r=r, single=single,
                final=(r == 2), dbg_out=outv,
            )

        nc.sync.dma_start(outv, vexp[:, :])


def _allreduce_squash(tc, dp, sp, prp, vp, repl, s_part, r, single=False, final=False, dbg_out=None):
    """AllReduce s [32,512] across cores, then v = squash(s).

    final=False: returns v replicated to 128 partitions as bf16 (vexp).
    final=True: returns v [32,512] f32 for the output DMA.
    """
    nc = tc.nc
    ccin = dp.tile([32, 512], F32, tag=f"ccin{r}")
    ccout = dp.tile([32, 512], F32, tag=f"ccout{r}")
    nc.sync.dma_start(ccin[:, :], s_part[:, :])
    if single:
        nc.sync.dma_start(ccout[:, :], ccin[:, :])
    else:
        nc.gpsimd.collective_compute(
            "AllReduce",
            mybir.AluOpType.add,
            replica_groups=[list(range(NCORES))],
            ins=[ccin[:, :].opt()],
            outs=[ccout[:, :].opt()],
        )
    s = sp.tile([32, 512], F32, tag="s_full")
    nc.sync.dma_start(s[:, :], ccout[:, :])
    if os.environ.get("DBG_S") == str(r):
        nc.sync.dma_start(dbg_out, s[:, :])

    # squash: v = s * (|s|^2/(1+|s|^2)/sqrt(|s|^2+eps)) per (b,j), |.| over t
    sq = sp.tile([32, 512], F32, tag="sq")
    nc.vector.tensor_mul(sq[:, :], s[:, :], s[:, :])
    ssq = sp.tile([32, 32], F32, tag="ssq")
    nc.vector.tensor_reduce(
        ssq[:, :],
        sq[:, :].rearrange("p (t j) -> p j t", t=16),
        mybir.AxisListType.X,
        mybir.AluOpType.add,
    )
    t1 = sp.tile([32, 32], F32, tag="t1")
    nc.vector.tensor_scalar_add(t1[:, :], ssq[:, :], 1.0)
    r1 = sp.tile([32, 32], F32, tag="r1")
    nc.vector.reciprocal(r1[:, :], t1[:, :])
    ssqe = sp.tile([32, 32], F32, tag="ssqe")
    nc.vector.tensor_scalar_add(ssqe[:, :], ssq[:, :], EPS)
    t2 = sp.tile([32, 32], F32, tag="t2")
    nc.scalar.activation(
        t2[:, :], ssqe[:, :], mybir.ActivationFunctionType.Sqrt, bias=0.0
    )
    r2 = sp.tile([32, 32], F32, tag="r2")
    nc.vector.reciprocal(r2[:, :], t2[:, :])
    sc = sp.tile([32, 32], F32, tag="sc")
    nc.vector.tensor_mul(sc[:, :], ssq[:, :], r1[:, :])
    nc.vector.tensor_mul(sc[:, :], sc[:, :], r2[:, :])

    scb = sc[:, :].unsqueeze(1).to_broadcast([32, 16, 32])
    if final:
        v = sp.tile([32, 512], F32, tag=f"v{r}")
        nc.vector.tensor_mul(
            v[:, :].rearrange("p (t j) -> p t j", t=16),
            s[:, :].rearrange("p (t j) -> p t j", t=16),
            scb,
        )
        return v

    vbf = sp.tile([32, 512], BF16, tag=f"vbf{r}")
    nc.vector.tensor_mul(
        vbf[:, :].rearrange("p (t j) -> p t j", t=16),
        s[:, :].rearrange("p (t j) -> p t j", t=16),
        scb,
    )
    # replicate to 128 partitions via PE: repl.T @ vbf
    rps = prp.tile([128, 512], F32, tag="repl")
    nc.tensor.matmul(
        rps[:, :], lhsT=repl[:, :], rhs=vbf[:, :], start=True, stop=True
    )
    vexp = vp.tile([128, 512], BF16, tag="vexp")
    nc.vector.tensor_copy(vexp[:, :], rps[:, :])
    return vexp


@functools.lru_cache(maxsize=2)
def _get_nc(single=False):
    return _build_program(single=single)


def _prep_inputs(inputs, W):
    """Build per-core input maps (host-side layout only)."""
    inputs = np.asarray(inputs, dtype=np.float32)
    W = np.asarray(W, dtype=np.float32)
    W0 = W[0]  # [J, I, T, D]

    # delta_b ones [K=(i_sub 4, b 32), M=(b' 32)]
    ones = np.zeros((4, 32, 32), dtype=np.float32)
    for b in range(32):
        ones[:, b, b] = 1.0
    ones = ones.reshape(128, 32).astype(ml_dtypes.bfloat16)

    # v replication matrix [K=b 32, M=(k4, b' 32)=128]
    repl = np.zeros((32, 4, 32), dtype=np.float32)
    for b in range(32):
        repl[b, :, b] = 1.0
    repl = repl.reshape(32, 128).astype(ml_dtypes.bfloat16)

    in_maps = []
    for c in range(NCORES):
        isl = slice(c * ILOC, (c + 1) * ILOC)
        ws = W0[:, isl]  # [J, 256, T, D]
        # wt[p, (gl, i_sub, d), (j, t)] ; i = (2p+gl)*4 + i_sub
        A = ws.transpose(1, 3, 0, 2)  # [i, d, j, t]
        A = A.reshape(NWT, 2, 4, D, J, T)  # p, gl, i_sub, d, j, t
        wtc = np.ascontiguousarray(A.reshape(NWT, 128, J * T))

        xs = inputs[:, isl]  # [b, 256, d]
        xt = xs.transpose(1, 2, 0)  # [i, d, b]
        xplc = np.ascontiguousarray(xt.reshape(NWT, 128, B))
        xt4 = xt.reshape(NWT, 2, 4, D, B)  # p, gl, i_sub, d, b
        xdc = np.zeros((NWT, 2, 4, D, 4, B), dtype=np.float32)
        ar = np.arange(4)
        # advanced indexing: result axes [i_sub, p, gl, d, b]
        xdc[:, :, ar, :, ar, :] = xt4.transpose(2, 0, 1, 3, 4)
        xdc = np.ascontiguousarray(xdc.reshape(NWT, 128, 128))

        in_maps.append(
            {
                "wt": wtc, "xd": xdc, "xplain": xplc, "onesdb": ones,
                "repl": repl, "warm": np.zeros((1, 8), dtype=np.float32),
            }
        )
    return in_maps


def kernel(inputs, W):
    import concourse.bass_utils as bass_utils

    nc = _get_nc()
    in_maps = _prep_inputs(inputs, W)
    res = bass_utils.run_bass_kernel_spmd(nc, in_maps, list(range(NCORES)))
    v = np.asarray(res.results[0]["outv"])  # [32, 512] = [b, (t, j)]
    return np.ascontiguousarray(
        v.reshape(B, T, J).transpose(0, 2, 1)
    ).astype(np.float32)
